# revision 8
# baseline (speedup 1.0000x reference)
"""LDA-loss logits kernel for Trainium2 (8 NeuronCores, SPMD).

Computes logits[b, c] = -0.5 * ||feat[b] - centers[c]||^2
                      = feat[b]·centers[c] - 0.5||feat[b]||^2 - 0.5||centers[c]||^2

Strategy (v3, fp8 DoubleRow):
  - Shard feat over batch: 4096 rows -> 512/core (4 m-tiles of 128), centers
    replicated.  Classes padded 10000 -> 10240 = 20 n-tiles of 512 so every
    matmul streams a full 512-wide moving operand (hides LDWEIGHTS).
  - Inputs quantized to fp8e4 on host; matmuls run perf_mode=DoubleRow
    (2 contraction chunks of 128 per pass -> ~1.8x bf16 column rate).  The
    squared-norm biases are host-precomputed fp32/fp16; fp8 error only
    touches the cross term (~6e-3 rel, tolerance 2e-2).
  - n-outer loop; center n-tiles stream in individually, and the first
    n-tile/feat are further split (per k-pair / per m-tile) so the first
    matmul issues after ~256KB of DMA instead of the full working set.
  - Eviction: ScalarE adds the per-row bias (fp32 PSUM -> fp16 SBUF),
    VectorE adds the per-column bias (fp16, 2x DVE rate).  The host upcasts
    the fp16 output after the gather (halves output HBM traffic).
  - All DMA on the sync HWDGE queue (SWDGE/gpsimd pays ~2us completion
    latency per store and a multi-us drain on the final ack); outputs are
    batched 4 m-tiles per store so the queue holds 20 output DMAs, not 80.
"""

import numpy as np
import ml_dtypes

BATCH = 4096
FEAT_DIM = 1024
NUM_CLASSES = 10000
N_CORES = 8
B_PER = BATCH // N_CORES            # 512 rows per core
P = 128
MT = B_PER // P                     # 4 output row tiles per core
KO = FEAT_DIM // P                  # 8 contraction chunks
KP = KO // 2                        # 4 DoubleRow chunk-pairs
NT = 20                             # n-tiles of 512
C_PAD = NT * 512                    # 10240 padded classes

_NC = None


def _build_bass():
    import concourse.mybir as mybir
    import concourse.tile as tile
    from concourse import bacc

    nc = bacc.Bacc("TRN2", target_bir_lowering=False, debug=False)

    featT = nc.dram_tensor("featT", [MT, P, KO * P], mybir.dt.float8e4,
                           kind="ExternalInput")
    centsT = nc.dram_tensor("centsT", [NT, P, KO * 512], mybir.dt.float8e4,
                            kind="ExternalInput")
    fsq = nc.dram_tensor("fsq", [P, MT], mybir.dt.float32, kind="ExternalInput")
    csq = nc.dram_tensor("csq", [NT // 4, P, 4 * 512], mybir.dt.float16,
                         kind="ExternalInput")
    out = nc.dram_tensor("out", [B_PER, C_PAD], mybir.dt.float16,
                         kind="ExternalOutput")

    with tile.TileContext(nc) as tc:
        _lda_tile_kernel(tc, featT.ap(), centsT.ap(), fsq.ap(), csq.ap(),
                         out.ap())
    nc.compile()
    return nc


def _lda_tile_kernel(tc, featT, centsT, fsq, csq, out):
    import concourse.mybir as mybir

    nc = tc.nc
    out_r = out.rearrange("(mo p) c -> p mo c", p=P)

    with (
        tc.tile_pool(name="big", bufs=1) as big,
        tc.tile_pool(name="consts", bufs=1) as consts,
        tc.tile_pool(name="ostage", bufs=6) as ostage,
        tc.tile_pool(name="psum", bufs=8, space="PSUM") as psum,
    ):
        cent_sb = big.tile([P, NT, KO, 512], mybir.dt.float8e4)
        feat_sb = big.tile([P, MT, KO, P], mybir.dt.float8e4)
        csq_sb = consts.tile([P, NT, 512], mybir.dt.float16)
        fsq_sb = consts.tile([P, MT], mybir.dt.float32)
        warm_sb = consts.tile([P, 192], mybir.dt.float8e4)

        # All input loads on the sync HWDGE queue in consumption order.  The
        # first matmul needs only feat m-tile 0 + the first k-pair of center
        # n-tile 0 (~256KB), so split those loads fine-grained; everything
        # later goes in n-tile-sized chunks that stay ahead of compute.
        # feat goes over the scalar HWDGE ring, centers over the sync ring:
        # the two rings generate descriptors in parallel, so the ~633ns/DMA
        # serial descriptor-gen cost doesn't stack up in front of the first
        # matmul.  (The scalar ring's first output store is ~14us in, well
        # after these four loads clear.)
        c0 = centsT[0].rearrange("p (ko c) -> p ko c", ko=KO)
        for m in range(MT):
            nc.scalar.dma_start(
                feat_sb[:, m], featT[m].rearrange("p (ko f) -> p ko f", ko=KO))
        for kp in range(KP):
            nc.sync.dma_start(cent_sb[:, 0, 2 * kp:2 * kp + 2],
                              c0[:, 2 * kp:2 * kp + 2])
        nc.scalar.dma_start(fsq_sb[:], fsq)
        c1 = centsT[1].rearrange("p (ko c) -> p ko c", ko=KO)
        nc.sync.dma_start(cent_sb[:, 1, 0:4], c1[:, 0:4])
        nc.sync.dma_start(cent_sb[:, 1, 4:KO], c1[:, 4:KO])
        for j in range(2, NT):
            nc.sync.dma_start(cent_sb[:, j],
                              centsT[j].rearrange("p (ko c) -> p ko c", ko=KO))
            if j % 4 == 2:
                b = j // 4
                nc.sync.dma_start(
                    csq_sb[:, 4 * b:4 * b + 4],
                    csq[b].rearrange("p (j c) -> p j c", j=4))

        # PE warm-up: ~6us of throwaway matmuls during the DMA prologue so
        # the HAM clock gate opens (1.2 -> 2.4 GHz takes ~3.4us of sustained
        # PE activity) before the first real matmul issues.
        nc.vector.memset(warm_sb[:], 0)
        warm_ps = psum.tile([P, 512], mybir.dt.float32, tag="ps", name="ps")
        for _ in range(56):
            nc.tensor.matmul(warm_ps[:, 0:64], warm_sb[:, 0:P],
                             warm_sb[:, P:P + 64], start=True, stop=True)

        for j in range(NT):
            ps = [psum.tile([P, 512], mybir.dt.float32, tag="ps", name="ps")
                  for _ in range(MT)]
            for kp in range(KP):
                for m in range(MT):
                    nc.tensor.matmul(
                        ps[m],
                        feat_sb[:, m, 2 * kp:2 * kp + 2, :],
                        cent_sb[:, j, 2 * kp:2 * kp + 2, :],
                        start=(kp == 0),
                        stop=(kp == KP - 1),
                        perf_mode=mybir.MatmulPerfMode.DoubleRow,
                    )
            ot = ostage.tile([P, MT, 512], mybir.dt.float16, tag="ot",
                             name="ot")
            for m in range(MT):
                # ot[m] = psum + fsq[row]  (per-partition bias on ScalarE)
                nc.scalar.activation(
                    ot[:, m], ps[m], mybir.ActivationFunctionType.Identity,
                    bias=fsq_sb[:, m:m + 1],
                )
                # ot[m] += csq[col]  (per-column bias on VectorE, fp16)
                nc.vector.tensor_add(ot[:, m], ot[:, m], csq_sb[:, j])
                if j == NT - 1:
                    # Final n-tile: store per m-tile from the sync ring
                    # (idle by now) so the last store pipelines with the
                    # remaining evictions instead of waiting for all four.
                    nc.sync.dma_start(
                        out_r[:, m, j * 512:(j + 1) * 512], ot[:, m])
            if j < NT - 1:
                # Output on the second HWDGE ring (Activation engine):
                # separate FIFO from the input ring, so stores never queue
                # behind loads.
                nc.scalar.dma_start(out_r[:, :, j * 512:(j + 1) * 512], ot)


def _get_nc():
    global _NC
    if _NC is None:
        _NC = _build_bass()
    return _NC


def _prep_inputs(feat, centers):
    feat = np.asarray(feat, dtype=np.float32)
    centers = np.asarray(centers, dtype=np.float32)
    f8 = ml_dtypes.float8_e4m3

    cent_pad = np.zeros((C_PAD, FEAT_DIM), dtype=np.float32)
    cent_pad[:NUM_CLASSES] = centers
    # centsT_sw[j, p, ko*512 + c] = centers[j*512 + c, ko*128 + p]
    centsT_sw = np.ascontiguousarray(
        cent_pad.T.astype(f8).reshape(KO, P, NT, 512).transpose(2, 1, 0, 3)
    ).reshape(NT, P, KO * 512)

    csq_v = np.zeros(C_PAD, dtype=np.float32)
    csq_v[:NUM_CLASSES] = -0.5 * np.einsum("cd,cd->c", centers, centers)
    csq_sw = np.ascontiguousarray(np.broadcast_to(
        csq_v.astype(np.float16).reshape(NT // 4, 1, 4 * 512),
        (NT // 4, P, 4 * 512)))

    feat8 = feat.astype(f8)
    fsq_v = -0.5 * np.einsum("bd,bd->b", feat, feat)

    in_maps = []
    for i in range(N_CORES):
        r0 = i * B_PER
        # featT_sw[mt, p, ko*128 + m] = feat[r0 + mt*128 + m, ko*128 + p]
        featT_sw = np.ascontiguousarray(
            feat8[r0:r0 + B_PER].T.reshape(KO, P, MT, P).transpose(2, 1, 0, 3)
        ).reshape(MT, P, KO * P)
        fsq_mat = np.ascontiguousarray(
            fsq_v[r0:r0 + B_PER].reshape(MT, P).T)
        in_maps.append({
            "featT": featT_sw,
            "centsT": centsT_sw,
            "fsq": fsq_mat,
            "csq": csq_sw,
        })
    return in_maps


def _run(inputs, trace=False, trace_cores=None):
    from concourse import bass_utils

    nc = _get_nc()
    in_maps = _prep_inputs(inputs["feat"], inputs["centers"])
    res = bass_utils.run_bass_kernel_spmd(
        nc, in_maps, core_ids=list(range(N_CORES)), trace=trace,
        trace_cores=trace_cores,
    )
    full = np.concatenate(
        [np.asarray(res.results[i]["out"]) for i in range(N_CORES)], axis=0)
    return full[:, :NUM_CLASSES].astype(np.float32), res


def kernel(**inputs) -> np.ndarray:
    return _run(inputs)[0]
